# revision 12
# baseline (speedup 1.0000x reference)
"""MoE feed-forward (top-2 of 8 experts, SwiGLU) on 8 Trainium2 NeuronCores.

Strategy (expert parallelism, per spec hint):
  - Launch 1 (data-parallel): each core computes router logits for T/8
    tokens with an fp32 PE matmul (router precision must match the fp32
    reference: min top2-vs-top3 logit gap is ~1e-4).
  - Host: top-2 + softmax over the two selected logits, build per-expert
    token lists, gather+transpose token activations per expert.
  - Launch 2 (expert-parallel): core e runs expert e's SwiGLU FFN over its
    gathered tokens (capacity-padded), scaling output rows by the combine
    weight on-device.
  - Host: scatter-add per-expert outputs back to token order.

All matmul FLOPs run on device; the host only reorders data.
"""

import os
import time as _time

import numpy as np

import concourse.bass as bass
import concourse.mybir as mybir
import concourse.tile as tile
from concourse.bass_utils import run_bass_kernel_spmd
from concourse.vector_clock import ScopedClock

F32 = mybir.dt.float32
F32R = mybir.dt.float32r
AF = mybir.ActivationFunctionType

B, S, D = 4, 1024, 1024
E, F, TOPK = 8, 2816, 2
T = B * S
NCORES = 8
TPC = T // NCORES          # router tokens per core
CAP = 1152                 # per-expert token capacity (measured max load 1071)
DK = D // 128              # 8 contraction chunks over D
FK = F // 128              # 22 chunks over F
CW = 384                   # phase-1 token-chunk width (3 x 384 = CAP)
MCH = CAP // 128           # 9 phase-2 token chunks
DT = D // 256              # 4 phase-2 output column tiles (256-wide: f32r full rate, quarter-sized wd tiles pipeline through the shared slot)

# FFN matmul operand dtype: float32 exact (4 cycles/row) or float32r
# (reduced-precision single-pass, 1 cycle/row at N>=256).
MM_DTYPE = F32R


class _TC(tile.TileContext):
    """Tail-drain workaround: this walrus build accepts only ONE sync-wait
    per CTRL instruction, but Tile's kernel-tail drain waits on every
    outstanding semaphore. Split it into a chain of single-wait drains."""

    def _drain_and_barrier(self, tick_clock, wait_clock):
        nc = self.nc
        drain_inst = nc.sync.drain()
        wait_clock.add_sem_waits(
            drain_inst.ins, ScopedClock({None: tick_clock.global_clock})
        )
        si = drain_inst.ins.sync_info
        waits = list(si.on_wait or [])
        if len(waits) > 1:
            si.on_wait = [waits[0]]
            for w in waits[1:]:
                d2 = nc.sync.drain()
                d2.ins.sync_info = mybir.SyncInfo(on_wait=[w], on_update=[])
        nc.all_engine_barrier()
        assert self.sems is not None
        popped = nc._tile_sem_poison_stack.pop()
        assert popped is self._sem_poison
        nc.clear_and_free_semaphores(list(self.sems.allocated().values()))
        nc.all_engine_barrier()


_nop_id = [0]


def _split_multi_waits(nc):
    """This walrus build accepts only one sync-wait command per instruction.
    Move extra waits onto single-wait NOPs inserted just before, on the same
    engine (engines dispatch in order, so the AND-semantics are preserved)."""
    from bass_rust import InstNoOp

    for fn in nc.m.functions:
        for blk in fn.blocks:
            insts = blk.instructions
            out = []
            changed = False
            for ins in insts:
                si = getattr(ins, "sync_info", None)
                waits = list(si.on_wait) if si is not None and si.on_wait else []
                if len(waits) > 1:
                    changed = True
                    for w in waits[:-1]:
                        _nop_id[0] += 1
                        nop = InstNoOp(name=f"I-waitnop-{_nop_id[0]}", ins=[], outs=[])
                        nop.engine = ins.engine
                        nop.sync_info = mybir.SyncInfo(on_wait=[w], on_update=[])
                        out.append(nop)
                    ins.sync_info = mybir.SyncInfo(
                        on_wait=[waits[-1]], on_update=list(si.on_update or [])
                    )
                out.append(ins)
            if changed:
                blk.instructions = out


def _mm(t):
    return t


def _router_prog():
    nc = bass.Bass()
    xr = nc.declare_dram_parameter("xr", [128, DK * TPC], F32, isOutput=False)
    rw = nc.declare_dram_parameter("rw", [128, DK * E], F32, isOutput=False)
    lg = nc.declare_dram_parameter("lgT", [E, TPC], F32, isOutput=True)
    with _TC(nc) as tc:
        with (
            tc.tile_pool(name="sb", bufs=1) as sb,
            tc.tile_pool(name="ps", bufs=1, space="PSUM") as ps,
        ):
            xs = sb.tile([128, DK * TPC], F32)
            for d in range(DK):
                nc.sync.dma_start(
                    xs[:, d * TPC : (d + 1) * TPC], xr[:, d * TPC : (d + 1) * TPC]
                )
            ws = sb.tile([128, DK * E], F32)
            nc.sync.dma_start(ws[:], rw[:])
            acc = ps.tile([E, TPC], F32)
            for d in range(DK):
                nc.tensor.matmul(
                    acc[:],
                    ws[:, d * E : (d + 1) * E],
                    xs[:, d * TPC : (d + 1) * TPC],
                    start=(d == 0),
                    stop=(d == DK - 1),
                )
            ot = sb.tile([E, TPC], F32)
            nc.vector.tensor_copy(ot[:], acc[:])
            nc.sync.dma_start(lg[:], ot[:])
    _split_multi_waits(nc)
    return nc


def _expert_prog():
    nc = bass.Bass()
    xe = nc.declare_dram_parameter("xe", [128, DK * CAP], MM_DTYPE, isOutput=False)
    wg = nc.declare_dram_parameter("wg", [FK, 128, DK * 128], MM_DTYPE, isOutput=False)
    wu = nc.declare_dram_parameter("wu", [FK, 128, DK * 128], MM_DTYPE, isOutput=False)
    wd = nc.declare_dram_parameter("wd", [DT, 128, FK * 256], MM_DTYPE, isOutput=False)
    sc = nc.declare_dram_parameter("sc", [128, MCH], F32, isOutput=False)
    ye = nc.declare_dram_parameter("ye", [CAP, D], F32, isOutput=True)

    with _TC(nc) as tc:
        with (
            # xs is dead after phase 1 and the phase-2 wd halves are the
            # same order of size -> share one tag-sized slot.
            tc.tile_pool(name="bigp", bufs=2) as bigp,
            tc.tile_pool(name="hres", bufs=1) as hres,
            tc.tile_pool(name="scp", bufs=1) as scp,
            tc.tile_pool(name="wgp", bufs=2) as wgp,
            tc.tile_pool(name="wup", bufs=2) as wup,
            tc.tile_pool(name="tmp", bufs=3) as tmp,
            tc.tile_pool(name="outp", bufs=3) as outp,
            tc.tile_pool(name="psg", bufs=2, space="PSUM") as psg,
            tc.tile_pool(name="psu", bufs=2, space="PSUM") as psu,
            tc.tile_pool(name="psy", bufs=3, space="PSUM") as psy,
        ):
            xs = bigp.tile([128, DK * CAP], MM_DTYPE, tag="big")
            for d in range(DK):
                nc.sync.dma_start(
                    xs[:, d * CAP : (d + 1) * CAP], xe[:, d * CAP : (d + 1) * CAP]
                )
            scs = scp.tile([128, MCH], F32)
            nc.sync.dma_start(scs[:], sc[:])
            hT = hres.tile([128, FK * CAP], MM_DTYPE)

            # Phase 1: hT[f*128+p, t] = silu(gate)[.] * up[.]  (F on partitions)
            for f in range(FK):
                wgt = wgp.tile([128, DK * 128], MM_DTYPE, tag="wgt")
                nc.sync.dma_start(wgt[:], wg[f])
                wut = wup.tile([128, DK * 128], MM_DTYPE, tag="wut")
                nc.sync.dma_start(wut[:], wu[f])
                for c in range(CAP // CW):
                    c0 = c * CW
                    pg = psg.tile([128, CW], F32, tag="pg")
                    pu = psu.tile([128, CW], F32, tag="pu")
                    for d in range(DK):
                        nc.tensor.matmul(
                            pg[:],
                            _mm(wgt[:, d * 128 : (d + 1) * 128]),
                            _mm(xs[:, d * CAP + c0 : d * CAP + c0 + CW]),
                            start=(d == 0),
                            stop=(d == DK - 1),
                        )
                    for d in range(DK):
                        nc.tensor.matmul(
                            pu[:],
                            _mm(wut[:, d * 128 : (d + 1) * 128]),
                            _mm(xs[:, d * CAP + c0 : d * CAP + c0 + CW]),
                            start=(d == 0),
                            stop=(d == DK - 1),
                        )
                    tg = tmp.tile([128, CW], F32, tag="tg")
                    nc.scalar.activation(tg[:], pg[:], AF.Silu)
                    nc.vector.tensor_mul(
                        hT[:, f * CAP + c0 : f * CAP + c0 + CW], tg[:], pu[:]
                    )

            # Phase 2: ye[t, :] = comb_weight[t] * (hT.T @ wd.T)
            for dt in range(DT):
                wdt = bigp.tile([128, FK * 256], MM_DTYPE, tag="big")
                nc.sync.dma_start(wdt[:], wd[dt])
                for m in range(MCH):
                    py = psy.tile([128, 256], F32, tag="py")
                    for f in range(FK):
                        nc.tensor.matmul(
                            py[:],
                            _mm(hT[:, f * CAP + m * 128 : f * CAP + (m + 1) * 128]),
                            _mm(wdt[:, f * 256 : (f + 1) * 256]),
                            start=(f == 0),
                            stop=(f == FK - 1),
                        )
                    ot = outp.tile([128, 256], F32, tag="ot")
                    nc.vector.tensor_scalar_mul(ot[:], py[:], scs[:, m : m + 1])
                    nc.sync.dma_start(
                        ye[m * 128 : (m + 1) * 128, dt * 256 : (dt + 1) * 256], ot[:]
                    )
    _split_multi_waits(nc)
    return nc


_progs = {}


def _get_progs():
    if "router" not in _progs:
        _progs["router"] = _router_prog()
        _progs["expert"] = _expert_prog()
    return _progs["router"], _progs["expert"]


def _dchunk_swizzle(a, inner):
    """[N, D] row-major -> [128, DK*inner] with out[p, d*inner + i] = a[i, d*128+p]."""
    n = a.shape[0]
    assert a.shape == (n, D) and inner == n
    return np.ascontiguousarray(a.reshape(n, DK, 128).transpose(2, 1, 0)).reshape(
        128, DK * n
    )


def _tick(msg, t0):
    if os.environ.get("KERNEL_TIMING"):
        print(f"  [kernel] {msg}: {_time.time()-t0:.3f}s", flush=True)
    return _time.time()


def kernel(x, router_w, w_gate, w_up, w_down):
    t0 = _time.time()
    x = np.asarray(x, np.float32)
    router_w = np.asarray(router_w, np.float32)
    w_gate = np.asarray(w_gate, np.float32)
    w_up = np.asarray(w_up, np.float32)
    w_down = np.asarray(w_down, np.float32)
    assert x.shape == (B, S, D)

    router_nc, expert_nc = _get_progs()
    t0 = _tick("get_progs", t0)
    xf = np.ascontiguousarray(x.reshape(T, D))

    # ---- Launch 1: router logits, data-parallel over tokens ----
    rw_h = np.ascontiguousarray(
        router_w.reshape(E, DK, 128).transpose(2, 1, 0)
    ).reshape(128, DK * E)
    in_maps = []
    for c in range(NCORES):
        xr_h = _dchunk_swizzle(xf[c * TPC : (c + 1) * TPC], TPC)
        in_maps.append({"xr": xr_h, "rw": rw_h})
    t0 = _tick("router prep", t0)
    rres = run_bass_kernel_spmd(router_nc, in_maps, list(range(NCORES)))
    t0 = _tick("router launch", t0)
    logits = np.concatenate([r["lgT"].T for r in rres.results], axis=0)  # [T, E]

    # ---- Host: top-2 + softmax + dispatch ----
    idx1 = np.argmax(logits, axis=1)
    l2 = logits.copy()
    l2[np.arange(T), idx1] = -np.inf
    idx2 = np.argmax(l2, axis=1)
    v1 = logits[np.arange(T), idx1]
    v2 = logits[np.arange(T), idx2]
    w1 = 1.0 / (1.0 + np.exp(v2 - v1))
    w2 = 1.0 - w1

    in_maps = []
    tok_lists = []
    for e in range(E):
        m1 = idx1 == e
        m2 = idx2 == e
        ids = np.concatenate([np.nonzero(m1)[0], np.nonzero(m2)[0]])
        wts = np.concatenate([w1[m1], w2[m2]]).astype(np.float32)
        ne = ids.shape[0]
        assert ne <= CAP, f"expert {e} over capacity: {ne} > {CAP}"
        tok_lists.append(ids)
        xtok = np.zeros((CAP, D), np.float32)
        xtok[:ne] = xf[ids]
        wts_p = np.zeros(CAP, np.float32)
        wts_p[:ne] = wts
        in_maps.append(
            {
                "xe": _dchunk_swizzle(xtok, CAP),
                "wg": np.ascontiguousarray(
                    w_gate[e].reshape(FK, 128, DK, 128).transpose(0, 3, 2, 1)
                ).reshape(FK, 128, DK * 128),
                "wu": np.ascontiguousarray(
                    w_up[e].reshape(FK, 128, DK, 128).transpose(0, 3, 2, 1)
                ).reshape(FK, 128, DK * 128),
                "wd": np.ascontiguousarray(
                    w_down[e].reshape(DT, 256, FK, 128).transpose(0, 3, 2, 1)
                ).reshape(DT, 128, FK * 256),
                "sc": np.ascontiguousarray(wts_p.reshape(MCH, 128).T),
            }
        )

    # ---- Launch 2: expert FFNs, expert-parallel ----
    t0 = _tick("dispatch prep", t0)
    eres = run_bass_kernel_spmd(expert_nc, in_maps, list(range(NCORES)))
    t0 = _tick("expert launch", t0)

    # ---- Host: combine (rows are pre-scaled on device) ----
    out = np.zeros((T, D), np.float32)
    for e in range(E):
        ids = tok_lists[e]
        out[ids] += eres.results[e]["ye"][: ids.shape[0]]
    _tick("combine", t0)
    return out.reshape(B, S, D)
